# revision 9
# baseline (speedup 1.0000x reference)
"""Trainium2 Bass kernel for gnn_message_passing (nn_Mesh1_14267881357850).

Reference computation (N=200000, D_SPATIAL=64, D_STRUCT=131, D_OUT=256):
    out1 = concat(spatial, structural) @ W_comb.T + b_comb          [N, 256]
    agg  = (structural + structural[neighbour].sum(1)) * 0.25       [N, 131]
    out2 = agg @ W_agg.T + b_agg                                    [N, 256]
returns (out1, out2)

Strategy (8 cores, node-parallel, bf16 compute):
  * Nodes padded to 200704, sharded 25088/core = 49 groups x 512 nodes,
    processed in 7 chunks of 7 groups (3584 nodes, 10752 edges).
  * Neighbour gathers use the SWDGE dma_gather ucode (~1us + 0.34ns/row per
    instruction) instead of indirect_dma_start (~1us per 128 rows):
      - structural is staged in DRAM as bf16 rows padded to 256 elems
        (512B, the dma_gather quantum), split into 7 banks of 32768 rows
        to satisfy the int16 index range.
      - per (chunk, bank): dense dma_gather calls (<=960 idxs each; the
        SWDGE descriptor ring caps one call at ~992 rows) pack that bank's
        edges into a staging tile at position i -> (lane i%128, slot i//128).
      - per 128-node subtile: one SBUF-source dma_gather (transpose=True,
        384 idxs; the transpose path caps at ~488) re-gathers the edge rows
        by (slot*128+lane) address, emitting them FEATURE-MAJOR in j-major
        node order -- replacing both the un-permutation and the PE
        transposes of the old design.
  * VectorE sums the 3 j-blocks in place and adds the (feature-major) self
    rows from a1A/a1B -> aggT; all matmuls run in bf16 (4x fp32 rate).
  * Activations feature-major from DRAM: a1A = structT[0:128] (shared by
    the out1 matmul and the agg self-add), a1B = [structT[128:131]; ones;
    spatialT]. Weight rows are reordered to match; 0.25 folded into W_agg.
  * Outputs accumulate in PSUM fp32, are copied to bf16, and stored as one
    [128, 4, 512] DMA per group into out[128, 4, npc] (host re-assembles).
"""

import os
import sys

import numpy as np
import ml_dtypes

for _p in ("/opt/trn_rl_repo", "/root/.axon_site/_ro/trn_rl_repo"):
    if os.path.isdir(_p) and _p not in sys.path:
        sys.path.append(_p)

import concourse.bacc as bacc
import concourse.bass as bass
import concourse.mybir as mybir
from concourse import library_config
from concourse.bass_utils import run_bass_kernel_spmd
from concourse.tile import TileContext

F32 = mybir.dt.float32
BF16 = mybir.dt.bfloat16
I16 = mybir.dt.int16

N = 200000
DS = 64          # spatial features
DT = 131         # structural features
DO = 256         # output features per head
NCORES = 8

GROUP = 512
SUBT = GROUP // 128          # 4 subtiles per group
NPC = 25088                  # nodes per core = 49 * 512
NG = NPC // GROUP            # 49 groups
CHUNK_G = 7                  # groups per chunk
NCHUNK = NG // CHUNK_G       # 7 chunks
CHUNK_N = CHUNK_G * GROUP    # 3584 nodes per chunk
CHUNK_E = 3 * CHUNK_N        # 10752 edges per chunk
NPAD = NPC * NCORES          # 200704

EROW = 256                   # padded structural row elems (512B bf16)
BANK = 32768                 # rows per gather bank (int16 index range)
NBANK = 7                    # ceil(200704 / 32768)
CALL_L = 960                 # idxs per bank-gather call (ring cap ~992)
CALL_SLOTS = 8               # roundup(960/128)
# calls per (chunk, bank): banks 0-5 get 2x960, short bank 6 gets 1x320.
# counts are Binomial(10752, 32768/200704): mean 1755 sd 38 (banks 0-5)
# and Binomial(10752, 3392/200000): mean 182 sd 13 (bank 6).
BANK_CALLS = [2] * 6 + [1]
LAST_L = 320
LAST_SLOTS = 3               # roundup(320/128)
LBANK = [2 * CALL_L] * 6 + [LAST_L]
SLOTS = [2 * CALL_SLOTS] * 6 + [LAST_SLOTS]
SLOT_BASE = np.concatenate([[0], np.cumsum(SLOTS)])[:NBANK]
TOT_SLOTS = int(np.sum(SLOTS))          # 99
BIDX_COLS = [2 * (CALL_L // 16)] * 6 + [LAST_L // 16]   # idx cols per bank
BIDX_OFF = np.concatenate([[0], np.cumsum(BIDX_COLS)])
BIDX_TOT = int(np.sum(BIDX_COLS))       # 740 per chunk
RIDX = 3 * 128               # regather idxs per subtile call
RIDX_COLS = RIDX // 16       # 24

KB = DS + 3 + 1              # 68 rows of a1B ([structT 128:131; ones; spatialT])

# exec time of the last traced run (ns), for test harnesses
last_exec_time_ns = None


def build_nc():
    nc = bacc.Bacc("TRN2", target_bir_lowering=False, debug=False)
    sfp = nc.dram_tensor("sfp", [NPAD, EROW], BF16, kind="ExternalInput")
    a1A = nc.dram_tensor("a1A", [128, NPC], BF16, kind="ExternalInput")
    a1B = nc.dram_tensor("a1B", [KB, NPC], BF16, kind="ExternalInput")
    bidx = nc.dram_tensor("bidx", [128, NCHUNK * BIDX_TOT], I16,
                          kind="ExternalInput")
    gidx = nc.dram_tensor("gidx", [128, NG * SUBT * RIDX_COLS], I16,
                          kind="ExternalInput")
    w1 = nc.dram_tensor("w1", [128 + KB, DO], BF16, kind="ExternalInput")
    w2 = nc.dram_tensor("w2", [128 + 4, DO], BF16, kind="ExternalInput")
    # out[p, b, n] = output feature (b*128 + p) of node n
    out = nc.dram_tensor("out", [128, 4, NPC], BF16, kind="ExternalOutput")

    with TileContext(nc) as tc:
        with (
            tc.tile_pool(name="const", bufs=1) as cpool,
            tc.tile_pool(name="act", bufs=2) as apool,
            tc.tile_pool(name="idx", bufs=2) as ipool,
            tc.tile_pool(name="stg", bufs=2) as spool,
            tc.tile_pool(name="rg", bufs=8) as rpool,
            tc.tile_pool(name="agg", bufs=3) as gpool,
            tc.tile_pool(name="osb", bufs=4) as opool,
            tc.tile_pool(name="ps", bufs=4, space="PSUM") as pspool,
        ):
            nc.gpsimd.load_library(library_config.mlp)

            w1a = cpool.tile([128, DO], BF16)
            nc.sync.dma_start(out=w1a, in_=w1[0:128, :])
            w1b = cpool.tile([KB, DO], BF16)
            nc.sync.dma_start(out=w1b, in_=w1[128 : 128 + KB, :])
            w2a = cpool.tile([128, DO], BF16)
            nc.sync.dma_start(out=w2a, in_=w2[0:128, :])
            w2b = cpool.tile([4, DO], BF16)
            nc.sync.dma_start(out=w2b, in_=w2[128:132, :])

            for ch in range(NCHUNK):
                c0 = ch * CHUNK_N

                bi = ipool.tile([128, BIDX_TOT], I16, tag="bi")
                nc.sync.dma_start(
                    out=bi, in_=bidx[:, ch * BIDX_TOT : (ch + 1) * BIDX_TOT])
                gcols = CHUNK_G * SUBT * RIDX_COLS
                gi = ipool.tile([128, gcols], I16, tag="gi")
                nc.sync.dma_start(
                    out=gi, in_=gidx[:, ch * gcols : (ch + 1) * gcols])
                aA = apool.tile([128, CHUNK_N], BF16, tag="aA")
                nc.sync.dma_start(out=aA, in_=a1A[:, c0 : c0 + CHUNK_N])
                aB = apool.tile([KB, CHUNK_N], BF16, tag="aB")
                nc.sync.dma_start(out=aB, in_=a1B[:, c0 : c0 + CHUNK_N])

                # ---- stage 1: dense banked DRAM gathers ----
                stg = spool.tile([128, TOT_SLOTS, EROW], BF16, tag="stg")
                for k in range(NBANK):
                    lo = k * BANK
                    hi = min((k + 1) * BANK, NPAD)
                    ncalls = BANK_CALLS[k]
                    for c in range(ncalls):
                        nidx = CALL_L if k < 6 else LAST_L
                        nslot = CALL_SLOTS if k < 6 else LAST_SLOTS
                        s0 = SLOT_BASE[k] + c * CALL_SLOTS
                        i0 = BIDX_OFF[k] + c * (CALL_L // 16)
                        nc.gpsimd.dma_gather(
                            out_ap=stg[:, s0 : s0 + nslot, :],
                            in_ap=sfp[lo:hi, :],
                            idxs_ap=bi[:, i0 : i0 + nidx // 16],
                            num_idxs=nidx,
                            num_idxs_reg=nidx,
                            elem_size=EROW,
                        )

                for gl in range(CHUNK_G):
                    g = ch * CHUNK_G + gl
                    n0 = g * GROUP

                    # ---- stage 2: per-subtile feature-major regathers ----
                    rgs = []
                    for b in range(SUBT):
                        i0 = (gl * SUBT + b) * RIDX_COLS
                        rg = rpool.tile([128, 2, RIDX], BF16, tag="rg")
                        rgs.append(rg)
                        nc.gpsimd.dma_gather(
                            out_ap=rg[:, :, :],
                            in_ap=stg[:, :, :],
                            idxs_ap=gi[:, i0 : i0 + RIDX_COLS],
                            num_idxs=RIDX,
                            num_idxs_reg=RIDX,
                            elem_size=EROW,
                            transpose=True,
                            sbuf_tokens_per_rank=128,
                            sbuf_free_dim_per_rank=EROW * 2,
                        )

                    # ---- neighbour sums + aggT assembly ----
                    aggA = gpool.tile([128, GROUP], BF16, tag="aggA")
                    aggB = gpool.tile([4, GROUP], BF16, tag="aggB")
                    nc.vector.memset(aggB[:, :], 1.0)
                    for b in range(SUBT):
                        rg = rgs[b]
                        bsl = slice(b * 128, (b + 1) * 128)
                        csl = slice(gl * GROUP + b * 128,
                                    gl * GROUP + (b + 1) * 128)
                        nc.vector.tensor_add(
                            out=rg[:, :, 0:128],
                            in0=rg[:, :, 0:128],
                            in1=rg[:, :, 128:256])
                        nc.vector.tensor_add(
                            out=rg[:, :, 0:128],
                            in0=rg[:, :, 0:128],
                            in1=rg[:, :, 256:384])
                        nc.vector.tensor_add(
                            out=aggA[:, bsl], in0=rg[:, 0, 0:128],
                            in1=aA[:, csl])
                        nc.vector.tensor_add(
                            out=aggB[0:3, bsl], in0=rg[0:3, 1, 0:128],
                            in1=aB[0:3, csl])

                    # ---- matmuls + store ----
                    gsl = slice(gl * GROUP, (gl + 1) * GROUP)
                    o = opool.tile([128, 4, GROUP], BF16, tag="o")
                    for c in range(2):
                        csl = slice(c * 128, (c + 1) * 128)
                        p1 = pspool.tile([128, GROUP], F32, tag="ps")
                        nc.tensor.matmul(
                            p1, lhsT=w1a[:, csl], rhs=aA[:, gsl],
                            start=True, stop=False)
                        nc.tensor.matmul(
                            p1, lhsT=w1b[:, csl], rhs=aB[:, gsl],
                            start=False, stop=True)
                        p2 = pspool.tile([128, GROUP], F32, tag="ps")
                        nc.tensor.matmul(
                            p2, lhsT=w2a[:, csl], rhs=aggA,
                            start=True, stop=False)
                        nc.tensor.matmul(
                            p2, lhsT=w2b[:, csl], rhs=aggB,
                            start=False, stop=True)
                        nc.scalar.copy(out=o[:, c, :], in_=p1)
                        nc.vector.tensor_copy(out=o[:, 2 + c, :], in_=p2)
                    nc.sync.dma_start(
                        out=out[:, :, n0 : n0 + GROUP], in_=o)
    nc.compile()
    return nc


def _wrap_idx(lst, ncols):
    """Flat list -> [128, ncols] int16: wrapped in 16 partitions
    (element i at partition i%16, col i//16), replicated to 8 blocks."""
    a = np.zeros(16 * ncols, dtype=np.int16)
    a[: len(lst)] = lst
    w = np.ascontiguousarray(a.reshape(ncols, 16).T)
    return np.tile(w, (8, 1))


def prep_inputs(spatial, structural, neighbour, W_agg, b_agg, W_comb, b_comb):
    spatial = np.asarray(spatial, dtype=np.float32)
    structural = np.asarray(structural, dtype=np.float32)
    nbr = np.asarray(neighbour, dtype=np.int64)

    pad = NPAD - N
    # gather source: bf16 rows padded to EROW elems
    sfp = np.zeros((NPAD, EROW), dtype=ml_dtypes.bfloat16)
    sfp[:N, :DT] = structural.astype(ml_dtypes.bfloat16)

    sp_p = np.concatenate([spatial, np.zeros((pad, DS), np.float32)], axis=0)
    st_p = np.concatenate([structural, np.zeros((pad, DT), np.float32)], axis=0)
    # spread pad nodes' (discarded) edges evenly so no bank list overflows
    nbr_fill = (np.arange(pad * 3, dtype=np.int64) * 104729 % N).reshape(pad, 3)
    nbr_p = np.concatenate([nbr, nbr_fill], axis=0)

    # weight row order must match [a1A; a1B] = [struct0:128; struct128:131; ones; spatial]
    Wc = np.asarray(W_comb, np.float32)   # [256, 64+131] cols: spatial then struct
    w1 = np.concatenate([
        Wc[:, DS : DS + 128].T,           # struct 0..127
        Wc[:, DS + 128 : DS + DT].T,      # struct 128..130
        np.asarray(b_comb, np.float32)[None, :],
        Wc[:, 0:DS].T,                    # spatial
    ], axis=0).astype(ml_dtypes.bfloat16)           # [196, 256]
    Wa = 0.25 * np.asarray(W_agg, np.float32)
    w2 = np.concatenate([
        Wa[:, 0:128].T,
        Wa[:, 128:DT].T,
        np.asarray(b_agg, np.float32)[None, :],
    ], axis=0).astype(ml_dtypes.bfloat16)           # [132, 256]

    in_maps = []
    for c in range(NCORES):
        sl = slice(c * NPC, (c + 1) * NPC)
        a1A = np.ascontiguousarray(
            st_p[sl, 0:128].T.astype(ml_dtypes.bfloat16))
        a1B = np.empty((KB, NPC), dtype=ml_dtypes.bfloat16)
        a1B[0:3] = st_p[sl, 128:DT].T
        a1B[3] = 1.0
        a1B[4:KB] = sp_p[sl].T

        nbr_c = nbr_p[sl].astype(np.int64)           # [NPC, 3]
        bidx_cols = []
        gidx_cols = []
        for ch in range(NCHUNK):
            flat = nbr_c[ch * CHUNK_N : (ch + 1) * CHUNK_N].reshape(-1)
            bank = flat >> 15
            pos = np.zeros(CHUNK_E, dtype=np.int64)
            for k in range(NBANK):
                m = bank == k
                cnt = int(m.sum())
                assert cnt <= LBANK[k], (c, ch, k, cnt)
                pos[m] = np.arange(cnt)
                bl = (flat[m] & 32767).astype(np.int16)
                bl = np.concatenate(
                    [bl, np.zeros(LBANK[k] - cnt, np.int16)])
                if k < 6:
                    bidx_cols.append(_wrap_idx(bl[:CALL_L], CALL_L // 16))
                    bidx_cols.append(_wrap_idx(bl[CALL_L:], CALL_L // 16))
                else:
                    bidx_cols.append(_wrap_idx(bl, LAST_L // 16))
            # slot address: bank base + per-call 8-slot block + in-call slot
            call = pos // CALL_L
            pic = pos - call * CALL_L                # position in call
            slot = SLOT_BASE[bank] + call * CALL_SLOTS + (pic >> 7)
            vaddr = ((slot << 7) | (pic & 127)).reshape(CHUNK_G, SUBT, 128, 3)
            for gl in range(CHUNK_G):
                for b in range(SUBT):
                    jm = vaddr[gl, b].T.reshape(-1).astype(np.int16)  # j-major
                    gidx_cols.append(_wrap_idx(jm, RIDX_COLS))
        in_maps.append({
            "sfp": sfp,
            "a1A": a1A,
            "a1B": a1B,
            "bidx": np.concatenate(bidx_cols, axis=1),
            "gidx": np.concatenate(gidx_cols, axis=1),
            "w1": w1,
            "w2": w2,
        })
    return in_maps


_NC_CACHE = {}


def kernel(spatial, structural, neighbour, W_agg, b_agg, W_comb, b_comb):
    global last_exec_time_ns
    if "nc" not in _NC_CACHE:
        _NC_CACHE["nc"] = build_nc()
    nc = _NC_CACHE["nc"]

    in_maps = prep_inputs(
        spatial, structural, neighbour, W_agg, b_agg, W_comb, b_comb)

    trace = bool(int(os.environ.get("KERNEL_TRACE", "0")))
    tmpdir = os.environ.get("KERNEL_TMPDIR") or None
    res = run_bass_kernel_spmd(
        nc, in_maps, core_ids=list(range(NCORES)), trace=trace, tmpdir=tmpdir)
    last_exec_time_ns = res.exec_time_ns

    # out[p, b, n] = feature (b*128+p) of node n; reassemble [512, N]
    comb = np.concatenate(
        [np.asarray(r["out"], dtype=np.float32).transpose(1, 0, 2)
         .reshape(512, NPC) for r in res.results], axis=1)[:, :N]
    out1 = np.ascontiguousarray(comb[:DO].T)
    out2 = np.ascontiguousarray(comb[DO:].T)
    return out1, out2


# revision 11
# speedup vs baseline: 1.7601x; 1.7601x over previous
"""Trainium2 Bass kernel for gnn_message_passing (nn_Mesh1_14267881357850).

Reference computation (N=200000, D_SPATIAL=64, D_STRUCT=131, D_OUT=256):
    out1 = concat(spatial, structural) @ W_comb.T + b_comb          [N, 256]
    agg  = (structural + structural[neighbour].sum(1)) * 0.25       [N, 131]
    out2 = agg @ W_agg.T + b_agg                                    [N, 256]
returns (out1, out2)

Strategy (8 cores, node-parallel, bf16 compute):
  * Nodes padded to 200704 and sharded 25088/core; `structural` is passed
    in full (bf16) to every core as the gather source.
  * The neighbour gather uses indirect_dma_start (hardware dynamic DMA:
    ~1us Q7 setup per instruction, then HW expands 128 descriptors --
    measured ~8.1ns/row, the cheapest gather on this part; the ucode
    dma_gather path runs ~9ns/row in software). One indirect DMA per
    (128-node subtile, neighbour slot) = 12 per 512-node group. This is
    the kernel's hard floor: ~588 instrs x ~1.04us on the Pool engine.
  * Everything else is sized to hide under that wall:
      - all activations, weights, transposes and matmuls in bf16
        (4x the fp32 matmul rate, 2x transpose rate);
      - VectorE sums the 3 neighbour rows node-major, PE transposes the
        sum into PSUM (bf16), VectorE adds the feature-major self rows;
      - per 128-node tile, 4 bf16 matmuls (K=128/68 for out1, K=128/4 for
        out2) accumulate [128, 512] fp32 PSUM tiles; ACT+DVE copy them to
        one bf16 SBUF tile; a single 3D DMA stores [128, 4, 512] per group;
      - a1T activations load in [*, 2048] slabs (4 groups per DMA).
  * Biases ride as a ones-row in a1T (out1) and a memset ones-row in the
    agg K=4 tile (out2); 0.25 is folded into W_agg host-side.
"""

import os
import sys

import numpy as np
import ml_dtypes

for _p in ("/opt/trn_rl_repo", "/root/.axon_site/_ro/trn_rl_repo"):
    if os.path.isdir(_p) and _p not in sys.path:
        sys.path.append(_p)

import concourse.bacc as bacc
import concourse.bass as bass
import concourse.mybir as mybir
from concourse.bass_utils import run_bass_kernel_spmd
from concourse.masks import make_identity
from concourse.tile import TileContext

F32 = mybir.dt.float32
BF16 = mybir.dt.bfloat16
I32 = mybir.dt.int32

N = 200000
DS = 64          # spatial features
DT = 131         # structural features
DO = 256         # output features per head
NCORES = 8
GROUP = 512      # nodes per pipeline group
SUBT = GROUP // 128   # 128-node subtiles per group
SLAB = 4         # groups per a1T load slab

NPC = 25088      # nodes per core (= 49 * 512)
NG = NPC // GROUP
NPAD = NPC * NCORES  # 200704

KA = DS + DT + 1     # 196 rows of a1T ([spatial; structural; ones])
KB = KA - 128        # 68

# exec time of the last traced run (ns), for test harnesses
last_exec_time_ns = None


def build_nc():
    nidx = 3 * SUBT              # indices per partition per group

    nc = bacc.Bacc("TRN2", target_bir_lowering=False, debug=False)
    a1T = nc.dram_tensor("a1T", [KA, NPC], BF16, kind="ExternalInput")
    sfull = nc.dram_tensor("sfull", [N, DT], BF16, kind="ExternalInput")
    idx = nc.dram_tensor("idx", [128, NG * nidx], I32, kind="ExternalInput")
    w1 = nc.dram_tensor("w1", [KA, DO], BF16, kind="ExternalInput")
    w2 = nc.dram_tensor("w2", [DT + 1, DO], BF16, kind="ExternalInput")
    # out[p, b, n] = output feature (b*128 + p) of node n
    out = nc.dram_tensor("out", [128, 4, NPC], BF16, kind="ExternalOutput")

    with TileContext(nc) as tc:
        with (
            tc.tile_pool(name="const", bufs=1) as cpool,
            tc.tile_pool(name="slab", bufs=2) as lpool,
            tc.tile_pool(name="nsums", bufs=40) as npool,
            tc.tile_pool(name="agg", bufs=3) as gpool,
            tc.tile_pool(name="osb", bufs=4) as opool,
            tc.tile_pool(name="pst", bufs=2, space="PSUM") as pst,
            tc.tile_pool(name="pout", bufs=4, space="PSUM") as pout,
        ):
            # ---- constants ----
            ident = cpool.tile([128, 128], BF16)
            make_identity(nc, ident)
            w1a = cpool.tile([128, DO], BF16)
            nc.sync.dma_start(out=w1a, in_=w1[0:128, :])
            w1b = cpool.tile([KB, DO], BF16)
            nc.sync.dma_start(out=w1b, in_=w1[128:KA, :])
            w2a = cpool.tile([128, DO], BF16)
            nc.sync.dma_start(out=w2a, in_=w2[0:128, :])
            w2b = cpool.tile([4, DO], BF16)
            nc.sync.dma_start(out=w2b, in_=w2[128 : DT + 1, :])
            idx_sb = cpool.tile([128, NG * nidx], I32)
            nc.sync.dma_start(out=idx_sb, in_=idx[:, :])

            for g in range(NG):
                n0 = g * GROUP

                # ---- a1T loads ----
                a1a = lpool.tile([128, GROUP], BF16, tag="a1a")
                nc.sync.dma_start(out=a1a, in_=a1T[0:128, n0 : n0 + GROUP])
                a1b = lpool.tile([KB, GROUP], BF16, tag="a1b")
                nc.sync.dma_start(out=a1b, in_=a1T[128:KA, n0 : n0 + GROUP])
                asl = slice(0, GROUP)

                # ---- indirect gathers: one DMA per (subtile, neighbour slot),
                # one offset per partition (HW dynamic-DMA limit). ----
                gts = []
                for b in range(SUBT):
                    row = []
                    base = (g * SUBT + b) * 3
                    for j in range(3):
                        g_t = npool.tile([128, DT], BF16, tag="gt")
                        row.append(g_t)
                        nc.gpsimd.indirect_dma_start(
                            out=g_t[:, :],
                            out_offset=None,
                            in_=sfull[:, :],
                            in_offset=bass.IndirectOffsetOnAxis(
                                ap=idx_sb[:, base + j : base + j + 1], axis=0
                            ),
                        )
                    gts.append(row)

                # ---- neighbour sum on VectorE, then PE transposes ----
                psA = pst.tile([128, GROUP], BF16, tag="psA")
                psB = pst.tile([3, GROUP], BF16, tag="psB")
                for b in range(SUBT):
                    nsum = npool.tile([128, DT], BF16, tag="nsum")
                    nc.vector.tensor_add(
                        out=nsum, in0=gts[b][0], in1=gts[b][1])
                    nc.vector.tensor_add(
                        out=nsum, in0=nsum, in1=gts[b][2])
                    nc.tensor.transpose(
                        psA[:, b * 128 : (b + 1) * 128],
                        nsum[:, 0:128],
                        ident,
                    )
                    nc.tensor.transpose(
                        psB[0:3, b * 128 : (b + 1) * 128],
                        nsum[:, 128:DT],
                        ident,
                    )

                # ---- aggT = nsumT + structT(self), feature-major ----
                # structural feats 0..63 live in a1a rows 64..127,
                # feats 64..127 in a1b rows 0..63, feats 128..130 in rows 64..66.
                aggA = gpool.tile([128, GROUP], BF16, tag="aggA")
                nc.vector.tensor_add(
                    out=aggA[0:64, :], in0=psA[0:64, :],
                    in1=a1a[64:128, asl])
                nc.vector.tensor_add(
                    out=aggA[64:128, :], in0=psA[64:128, :],
                    in1=a1b[0:64, asl])
                aggB = gpool.tile([4, GROUP], BF16, tag="aggB")
                # rows 0..2 overwritten below; row 3 stays 1.0 (bias ones-row)
                nc.vector.memset(aggB[:, :], 1.0)
                nc.vector.tensor_add(
                    out=aggB[0:3, :], in0=psB[0:3, :],
                    in1=a1b[64:67, asl])

                # ---- matmuls (weights stationary, bf16) + store ----
                o = opool.tile([128, 4, GROUP], BF16, tag="o")
                for c in range(2):
                    csl = slice(c * 128, (c + 1) * 128)
                    p1 = pout.tile([128, GROUP], F32, tag="ps")
                    nc.tensor.matmul(
                        p1, lhsT=w1a[:, csl], rhs=a1a[:, asl],
                        start=True, stop=False)
                    nc.tensor.matmul(
                        p1, lhsT=w1b[:, csl], rhs=a1b[:, asl],
                        start=False, stop=True)
                    p2 = pout.tile([128, GROUP], F32, tag="ps")
                    nc.tensor.matmul(
                        p2, lhsT=w2a[:, csl], rhs=aggA, start=True, stop=False)
                    nc.tensor.matmul(
                        p2, lhsT=w2b[:, csl], rhs=aggB, start=False, stop=True)
                    nc.scalar.copy(out=o[:, c, :], in_=p1)
                    nc.vector.tensor_copy(out=o[:, 2 + c, :], in_=p2)
                nc.sync.dma_start(out=out[:, :, n0 : n0 + GROUP], in_=o)
    nc.compile()
    return nc


def prep_inputs(spatial, structural, neighbour, W_agg, b_agg, W_comb, b_comb):
    """Host-side shard + layout transform. Returns list of per-core in_maps."""
    spatial = np.asarray(spatial, dtype=np.float32)
    structural = np.asarray(structural, dtype=np.float32)
    nbr = np.asarray(neighbour, dtype=np.int32)

    sfull = np.ascontiguousarray(structural.astype(ml_dtypes.bfloat16))

    pad = NPAD - N
    spatial_p = np.concatenate(
        [spatial, np.zeros((pad, DS), np.float32)], axis=0)
    structural_p = np.concatenate(
        [structural, np.zeros((pad, DT), np.float32)], axis=0)
    nbr_p = np.concatenate([nbr, np.zeros((pad, 3), np.int32)], axis=0)

    w1 = np.concatenate(
        [np.asarray(W_comb, np.float32).T,
         np.asarray(b_comb, np.float32)[None, :]], axis=0)
    w1 = np.ascontiguousarray(w1).astype(ml_dtypes.bfloat16)    # [196, 256]
    w2 = np.concatenate(
        [0.25 * np.asarray(W_agg, np.float32).T,
         np.asarray(b_agg, np.float32)[None, :]], axis=0)
    w2 = np.ascontiguousarray(w2).astype(ml_dtypes.bfloat16)    # [132, 256]

    in_maps = []
    for c in range(NCORES):
        sl = slice(c * NPC, (c + 1) * NPC)
        a1T = np.empty((KA, NPC), ml_dtypes.bfloat16)
        a1T[0:DS] = spatial_p[sl].T
        a1T[DS : DS + DT] = structural_p[sl].T
        a1T[DS + DT] = 1.0
        # idx[p, (g*SUBT + b)*3 + j] = nbr[c*NPC + g*GROUP + b*128 + p, j]
        ngt = NPC // 128
        idx = np.ascontiguousarray(
            nbr_p[sl].reshape(ngt, 128, 3)
            .transpose(1, 0, 2).reshape(128, ngt * 3))
        in_maps.append({
            "a1T": a1T,
            "sfull": sfull,
            "idx": idx,
            "w1": w1,
            "w2": w2,
        })
    return in_maps


_NC_CACHE = {}


def kernel(spatial, structural, neighbour, W_agg, b_agg, W_comb, b_comb):
    global last_exec_time_ns
    if "nc" not in _NC_CACHE:
        _NC_CACHE["nc"] = build_nc()
    nc = _NC_CACHE["nc"]

    in_maps = prep_inputs(
        spatial, structural, neighbour, W_agg, b_agg, W_comb, b_comb)

    trace = bool(int(os.environ.get("KERNEL_TRACE", "0")))
    tmpdir = os.environ.get("KERNEL_TMPDIR") or None
    res = run_bass_kernel_spmd(
        nc, in_maps, core_ids=list(range(NCORES)), trace=trace, tmpdir=tmpdir)
    last_exec_time_ns = res.exec_time_ns

    # out[p, b, n] = feature (b*128+p) of node n; reassemble [512, N]
    comb = np.concatenate(
        [np.asarray(r["out"], dtype=np.float32).transpose(1, 0, 2)
         .reshape(512, NPC) for r in res.results], axis=1)[:, :N]
    out1 = np.ascontiguousarray(comb[:DO].T)
    out2 = np.ascontiguousarray(comb[DO:].T)
    return out1, out2
